# revision 1
# baseline (speedup 1.0000x reference)
"""JIIF-style implicit-upsampling MLP on 8 Trainium2 NeuronCores.

Full inputs -> shard (batch, query-point) across 8 cores -> per-core Bass/Tile
kernel (gather + 5-layer MLP + softmax-combine) -> gather full output.

Per-core kernel design (NP = 32768 query points/core):
  * DRAM tables per core:  tbl_fl [4097, 256]  = concat(feat, lr_guide) per
    LR pixel + zero row (row 4096) for out-of-range zero padding;
    tbl_hr_lo/hi [32768, 128] = hr_guide per HR pixel, split in two halves
    because dma_gather indices are int16 (invalid hr points are zeroed via a
    mask multiply instead of a zero row).
  * On-device index math reproduces grid_sample_nearest bit-exactly
    (round-half-even via the 1.5*2^23 magic-add; invalid points redirected
    to the zero row / masked).
  * Gathers via gpsimd dma_gather, 512 rows per call (row i lands on
    partition i%128, slot i//128 — exactly the point-major layout); indices
    are int16 in the [16, n/16] wrap replicated across the 8 Q7 groups.
    (indirect_dma_start was abandoned: its compile-time DynamicDMA patching
    is disabled in this toolchain and returns wrong rows on HW.)
  * Gathered [pts, ch] tiles are PE-transposed to [ch, pts].
  * MLP (386-1024-512-256-128-2) as float32r matmuls, N=512 points per tile,
    PSUM K-accumulation, fused bias+ReLU evacuation alternating DVE/ACT.
  * Final layer uses activations as the stationary operand to produce
    [pts, 2] directly in PSUM; softmax over the 4 shifts + weighted sum is
    done in point-major layout on DVE/ACT.
"""
import sys

if "/opt/trn_rl_repo" not in sys.path:
    sys.path.insert(0, "/opt/trn_rl_repo")

import numpy as np

import concourse.bass as bass
import concourse.bacc as bacc
import concourse.tile as tile
from concourse import mybir
from concourse.masks import make_identity

F32 = mybir.dt.float32
F32R = mybir.dt.float32r
I32 = mybir.dt.int32
I16 = mybir.dt.int16
OP = mybir.AluOpType
ACTF = mybir.ActivationFunctionType
AX = mybir.AxisListType

MAGIC = 12582912.0  # 1.5 * 2**23 : forces round-to-nearest-even on f32 add

B, NFULL = 4, 65536
H_LR = 64
H_HR = 256
NCORES = 8
NP = (B * NFULL) // NCORES  # 32768 points per core
PIX_FL = H_LR * H_LR        # 4096
PIX_HR = H_HR * H_HR        # 65536
SHIFTS = [(-1.0 / 64, -1.0 / 64), (-1.0 / 64, 1.0 / 64),
          (1.0 / 64, -1.0 / 64), (1.0 / 64, 1.0 / 64)]


def build_program(npoints=NP):
    """Build the per-core Bass program. npoints must be a multiple of 512."""
    assert npoints % 512 == 0
    NQ = npoints // 128          # free-dim length of point-major tiles
    T = NQ // 4                  # number of 512-point tiles

    nc = bacc.Bacc("TRN2", target_bir_lowering=False, debug=False)

    tbl_fl = nc.dram_tensor("tbl_fl", [PIX_FL + 1, 256], F32, kind="ExternalInput")
    tbl_hr_lo = nc.dram_tensor("tbl_hr_lo", [PIX_HR // 2, 128], F32, kind="ExternalInput")
    tbl_hr_hi = nc.dram_tensor("tbl_hr_hi", [PIX_HR // 2, 128], F32, kind="ExternalInput")
    coord = nc.dram_tensor("coord", [npoints, 2], F32, kind="ExternalInput")
    w0a = nc.dram_tensor("w0a", [128, 1024], F32R, kind="ExternalInput")
    w0b = nc.dram_tensor("w0b", [128, 1024], F32R, kind="ExternalInput")
    w0c = nc.dram_tensor("w0c", [128, 1024], F32R, kind="ExternalInput")
    w0d = nc.dram_tensor("w0d", [2, 1024], F32R, kind="ExternalInput")
    w1 = nc.dram_tensor("w1", [128, 4096], F32R, kind="ExternalInput")
    w2 = nc.dram_tensor("w2", [128, 1024], F32R, kind="ExternalInput")
    w3 = nc.dram_tensor("w3", [128, 256], F32R, kind="ExternalInput")
    w4 = nc.dram_tensor("w4", [128, 2], F32R, kind="ExternalInput")
    bias0 = nc.dram_tensor("bias0", [128, 8], F32, kind="ExternalInput")
    bias1 = nc.dram_tensor("bias1", [128, 4], F32, kind="ExternalInput")
    bias2 = nc.dram_tensor("bias2", [128, 2], F32, kind="ExternalInput")
    bias3 = nc.dram_tensor("bias3", [128, 1], F32, kind="ExternalInput")
    bias4 = nc.dram_tensor("bias4", [128, 1], F32, kind="ExternalInput")
    out = nc.dram_tensor("out", [npoints], F32, kind="ExternalOutput")

    evac_ctr = [0]

    def evac_relu(dst, src, bias_ap):
        # relu(src + bias), alternating DVE / ACT to balance engine load
        if evac_ctr[0] % 2 == 0:
            nc.vector.tensor_scalar(dst, src, bias_ap, 0.0, OP.add, OP.max)
        else:
            nc.scalar.activation(dst, src, ACTF.Relu, bias=bias_ap, scale=1.0)
        evac_ctr[0] += 1

    def evac_copy(dst, src):
        if evac_ctr[0] % 2 == 0:
            nc.vector.tensor_copy(dst, src)
        else:
            nc.scalar.copy(dst, src)
        evac_ctr[0] += 1

    with tile.TileContext(nc) as tc:
        with tc.tile_pool(name="const", bufs=1) as cp, \
             tc.tile_pool(name="prol", bufs=1) as pp, \
             tc.tile_pool(name="gat", bufs=3) as gp, \
             tc.tile_pool(name="rhs", bufs=3) as rp, \
             tc.tile_pool(name="act", bufs=2) as ap, \
             tc.tile_pool(name="sm", bufs=2) as smp, \
             tc.tile_pool(name="ps", bufs=1, space="PSUM") as ps:

            ident = cp.tile([128, 128], F32)
            make_identity(nc, ident[:])

            # ---- load weights / biases ----
            w0a_s = cp.tile([128, 1024], F32R)
            w0b_s = cp.tile([128, 1024], F32R)
            w0c_s = cp.tile([128, 1024], F32R)
            w0d_s = cp.tile([2, 1024], F32R)
            w1_s = cp.tile([128, 4096], F32R)
            w2_s = cp.tile([128, 1024], F32R)
            w3_s = cp.tile([128, 256], F32R)
            w4_s = cp.tile([128, 2], F32R)
            b0_s = cp.tile([128, 8], F32)
            b1_s = cp.tile([128, 4], F32)
            b2_s = cp.tile([128, 2], F32)
            b3_s = cp.tile([128, 1], F32)
            b4_s = cp.tile([128, 1], F32)
            for dst, src in [(w0a_s, w0a), (w0b_s, w0b), (w0c_s, w0c),
                             (w0d_s, w0d), (w1_s, w1), (w2_s, w2), (w3_s, w3),
                             (w4_s, w4), (b0_s, bias0), (b1_s, bias1),
                             (b2_s, bias2), (b3_s, bias3), (b4_s, bias4)]:
                nc.sync.dma_start(dst[:], src[:])

            # ---- load coords: point n -> (partition n%128, free n//128) ----
            C = pp.tile([128, NQ, 2], F32)
            nc.sync.dma_start(C[:], coord[:].rearrange("(q p) t -> p q t", p=128))

            # ---- index math ----
            def axis_index(c_ap, shift, Hval, tag):
                """Returns (rc, m): clipped rounded index + valid mask, [128, NQ] f32."""
                t0 = pp.tile([128, NQ], F32, tag="ax_t")
                if shift is not None:
                    nc.vector.tensor_scalar(t0[:], c_ap, shift, None, OP.add)
                    src = t0[:]
                else:
                    src = c_ap
                v = pp.tile([128, NQ], F32, tag="ax_v")
                nc.vector.tensor_scalar(v[:], src, 1.0, float(Hval), OP.add, OP.mult)
                nc.vector.tensor_scalar(v[:], v[:], 1.0, 0.5, OP.subtract, OP.mult)
                r = pp.tile([128, NQ], F32, tag="ax_r")
                nc.vector.tensor_scalar(r[:], v[:], MAGIC, MAGIC, OP.add, OP.subtract)
                rc = pp.tile([128, NQ], F32, tag=tag[-1] + "_rc")
                nc.vector.tensor_scalar(rc[:], r[:], 0.0, float(Hval - 1), OP.max, OP.min)
                m = pp.tile([128, NQ], F32, tag=tag[-1] + "_m")
                nc.vector.tensor_tensor(m[:], r[:], rc[:], OP.is_equal)
                return rc, m

            def lin_index(ry, rx, my, mx, Hval, tag, redirect=True):
                """Combined mask + linear index; optionally redirected to the
                zero row (Hval^2) when invalid."""
                m = pp.tile([128, NQ], F32, tag="li_mm")
                nc.vector.tensor_tensor(m[:], my[:], mx[:], OP.mult)
                idx = pp.tile([128, NQ], F32, tag="li_idx")
                nc.vector.scalar_tensor_tensor(idx[:], ry[:], float(Hval), rx[:],
                                               OP.mult, OP.add)
                if redirect:
                    zr = float(Hval * Hval)
                    nc.vector.scalar_tensor_tensor(idx[:], idx[:], -zr, m[:],
                                                   OP.add, OP.mult)
                    nc.vector.tensor_scalar(idx[:], idx[:], zr, None, OP.add)
                return idx, m

            def wrap16(src_i16, tag):
                # relayout [128, NQ] -> dma_gather's [16, NQ*8] wrap,
                # replicated across the 8 Q7 partition groups
                wr = pp.tile([128, NQ * 8], I16, tag=tag + "_wr")
                for ph in range(8):
                    nc.sync.dma_start(wr[0:16, ph::8],
                                      src_i16[ph * 16:(ph + 1) * 16, :])
                for rep in range(1, 8):
                    nc.sync.dma_start(wr[rep * 16:(rep + 1) * 16, :], wr[0:16, :])
                return wr

            cy, cx = C[:, :, 0], C[:, :, 1]

            ry_h, my_h = axis_index(cy, None, H_HR, "hy")
            rx_h, mx_h = axis_index(cx, None, H_HR, "hx")
            idx_hf, m_hr = lin_index(ry_h, rx_h, my_h, mx_h, H_HR, "h",
                                     redirect=False)
            HALF = float(PIX_HR // 2)
            hi_m = pp.tile([128, NQ], F32)
            nc.vector.tensor_scalar(hi_m[:], idx_hf[:], HALF, None, OP.is_ge)
            one_m_hi = pp.tile([128, NQ], F32)
            nc.vector.tensor_scalar(one_m_hi[:], hi_m[:], -1.0, 1.0, OP.mult, OP.add)
            ilo = pp.tile([128, NQ], F32)
            nc.vector.tensor_tensor(ilo[:], idx_hf[:], one_m_hi[:], OP.mult)
            ihi = pp.tile([128, NQ], F32)
            nc.vector.scalar_tensor_tensor(ihi[:], idx_hf[:], -HALF, hi_m[:],
                                           OP.add, OP.mult)
            mlo_m = pp.tile([128, NQ], F32)
            nc.vector.tensor_tensor(mlo_m[:], one_m_hi[:], m_hr[:], OP.mult)
            mhi_m = pp.tile([128, NQ], F32)
            nc.vector.tensor_tensor(mhi_m[:], hi_m[:], m_hr[:], OP.mult)
            ilo16 = pp.tile([128, NQ], I16)
            nc.vector.tensor_copy(ilo16[:], ilo[:])
            ihi16 = pp.tile([128, NQ], I16)
            nc.vector.tensor_copy(ihi16[:], ihi[:])
            wr_hlo = wrap16(ilo16, "hlo")
            wr_hhi = wrap16(ihi16, "hhi")

            idx_fl = []
            rel = []
            for s, (sy, sx) in enumerate(SHIFTS):
                ry, my = axis_index(cy, sy, H_LR, "fy")
                rx, mx = axis_index(cx, sx, H_LR, "fx")
                fidx, m = lin_index(ry, rx, my, mx, H_LR, f"f{s}")
                f16 = pp.tile([128, NQ], I16, tag="f16")
                nc.vector.tensor_copy(f16[:], fidx[:])
                idx_fl.append(wrap16(f16, f"fw{s}"))
                # rel_coord = (coord - valid*pix_coord) * 64, bit-exact vs ref
                rl = pp.tile([128, NQ, 2], F32, tag=f"rel{s}")
                for comp, (rc_c, c_c) in enumerate([(ry, cy), (rx, cx)]):
                    qc = pp.tile([128, NQ], F32, tag="qc")
                    nc.vector.tensor_scalar(qc[:], rc_c[:], 0.03125, -0.984375,
                                            OP.mult, OP.add)
                    nc.vector.tensor_tensor(qc[:], qc[:], m[:], OP.mult)
                    nc.vector.tensor_tensor(qc[:], c_c, qc[:], OP.subtract)
                    nc.vector.tensor_scalar(rl[:, :, comp], qc[:], 64.0, None, OP.mult)
                rel.append(rl)

            out_sb = pp.tile([128, NQ], F32)

            # ---- main loop over 512-point tiles ----
            for t in range(T):
                q4 = slice(t * 4, t * 4 + 4)

                # hr gather + transpose -> hrT [128ch, 512pts]
                gh = gp.tile([128, 4, 128], F32, tag="gh")
                ghi = gp.tile([128, 4, 128], F32, tag="ghi")
                w32 = slice(t * 32, (t + 1) * 32)
                nc.gpsimd.dma_gather(gh[:], tbl_hr_lo[:], wr_hlo[:, w32],
                                     num_idxs=512, num_idxs_reg=512, elem_size=128)
                nc.gpsimd.dma_gather(ghi[:], tbl_hr_hi[:], wr_hhi[:, w32],
                                     num_idxs=512, num_idxs_reg=512, elem_size=128)
                mlo_b = mlo_m[:, q4].unsqueeze(2).to_broadcast([128, 4, 128])
                mhi_b = mhi_m[:, q4].unsqueeze(2).to_broadcast([128, 4, 128])
                nc.vector.tensor_tensor(gh[:], gh[:], mlo_b, OP.mult)
                nc.vector.tensor_tensor(ghi[:], ghi[:], mhi_b, OP.mult)
                nc.vector.tensor_tensor(gh[:], gh[:], ghi[:], OP.add)
                pt_h = ps.tile([128, 512], F32, tag="pt", bufs=3)
                for q in range(4):
                    nc.tensor.transpose(pt_h[:, q * 128:(q + 1) * 128],
                                        gh[:, q, :], ident[:])
                hrT = rp.tile([128, 512], F32R, tag="hrT")
                evac_copy(hrT[:], pt_h[:])

                p4 = ps.tile([128, 32], F32, tag="p4", bufs=2)

                for s in range(4):
                    gfl = gp.tile([128, 4, 256], F32, tag="gfl")
                    nc.gpsimd.dma_gather(gfl[:], tbl_fl[:], idx_fl[s][:, w32],
                                         num_idxs=512, num_idxs_reg=512,
                                         elem_size=256)

                    pt_f = ps.tile([128, 512], F32, tag="pt", bufs=3)
                    pt_l = ps.tile([128, 512], F32, tag="pt", bufs=3)
                    for q in range(4):
                        nc.tensor.transpose(pt_f[:, q * 128:(q + 1) * 128],
                                            gfl[:, q, 0:128], ident[:])
                        nc.tensor.transpose(pt_l[:, q * 128:(q + 1) * 128],
                                            gfl[:, q, 128:256], ident[:])
                    featT = rp.tile([128, 512], F32R, tag="featT")
                    lrT = rp.tile([128, 512], F32R, tag="lrT")
                    evac_copy(featT[:], pt_f[:])
                    evac_copy(lrT[:], pt_l[:])

                    pt_r = ps.tile([2, 512], F32, tag="pt", bufs=3)
                    for q in range(4):
                        nc.tensor.transpose(pt_r[:, q * 128:(q + 1) * 128],
                                            rel[s][:, t * 4 + q, :], ident[:])
                    relT = rp.tile([2, 512], F32R, tag="relT")
                    evac_copy(relT[:], pt_r[:])

                    # L0: 386 -> 1024
                    a0 = ap.tile([128, 8, 512], F32R, tag="a0", bufs=1)
                    for m in range(8):
                        ms = slice(m * 128, (m + 1) * 128)
                        p0 = ps.tile([128, 512], F32, tag="pmm", bufs=3)
                        nc.tensor.matmul(p0[:], w0a_s[:, ms], featT[:],
                                         start=True, stop=False)
                        nc.tensor.matmul(p0[:], w0b_s[:, ms], hrT[:],
                                         start=False, stop=False)
                        nc.tensor.matmul(p0[:], w0c_s[:, ms], lrT[:],
                                         start=False, stop=False)
                        nc.tensor.matmul(p0[:], w0d_s[:, ms], relT[:],
                                         start=False, stop=True)
                        evac_relu(a0[:, m, :], p0[:], b0_s[:, m:m + 1])

                    # L1: 1024 -> 512
                    a1 = ap.tile([128, 4, 512], F32R, tag="a1")
                    for m in range(4):
                        p1 = ps.tile([128, 512], F32, tag="pmm", bufs=3)
                        for k in range(8):
                            nc.tensor.matmul(
                                p1[:],
                                w1_s[:, k * 512 + m * 128: k * 512 + (m + 1) * 128],
                                a0[:, k, :],
                                start=(k == 0), stop=(k == 7))
                        evac_relu(a1[:, m, :], p1[:], b1_s[:, m:m + 1])

                    # L2: 512 -> 256
                    a2 = ap.tile([128, 2, 512], F32R, tag="a2")
                    for m in range(2):
                        p2 = ps.tile([128, 512], F32, tag="pmm", bufs=3)
                        for k in range(4):
                            nc.tensor.matmul(
                                p2[:],
                                w2_s[:, k * 256 + m * 128: k * 256 + (m + 1) * 128],
                                a1[:, k, :],
                                start=(k == 0), stop=(k == 3))
                        evac_relu(a2[:, m, :], p2[:], b2_s[:, m:m + 1])

                    # L3: 256 -> 128
                    a3 = ap.tile([128, 512], F32R, tag="a3")
                    p3 = ps.tile([128, 512], F32, tag="pmm", bufs=3)
                    for k in range(2):
                        nc.tensor.matmul(p3[:],
                                         w3_s[:, k * 128:(k + 1) * 128],
                                         a2[:, k, :],
                                         start=(k == 0), stop=(k == 1))
                    evac_relu(a3[:], p3[:], b3_s[:, 0:1])

                    # L4: 128 -> 2, activations stationary -> [pts, 2] in PSUM
                    for q in range(4):
                        off = (q * 4 + s) * 2
                        nc.tensor.matmul(p4[:, off:off + 2],
                                         a3[:, q * 128:(q + 1) * 128],
                                         w4_s[:],
                                         start=True, stop=True)

                # softmax over shifts + weighted sum (point-major layout)
                p4v = p4[:].rearrange("p (q s c) -> p q s c", q=4, s=4)
                mx = smp.tile([128, 4], F32, tag="mx")
                nc.vector.tensor_reduce(mx[:], p4v[:, :, :, 1], AX.X, OP.max)
                e = smp.tile([128, 4, 4], F32, tag="e")
                mxb = mx[:].unsqueeze(2).to_broadcast([128, 4, 4])
                nc.vector.tensor_tensor(e[:], p4v[:, :, :, 1], mxb, OP.subtract)
                nc.scalar.activation(e[:], e[:], ACTF.Exp)
                ssum = smp.tile([128, 4], F32, tag="ssum")
                nc.vector.tensor_reduce(ssum[:], e[:], AX.X, OP.add)
                nc.vector.tensor_tensor(e[:], e[:], p4v[:, :, :, 0], OP.mult)
                num = smp.tile([128, 4], F32, tag="num")
                nc.vector.tensor_reduce(num[:], e[:], AX.X, OP.add)
                rec = smp.tile([128, 4], F32, tag="rec")
                nc.vector.reciprocal(rec[:], ssum[:])
                nc.vector.tensor_tensor(num[:], num[:], rec[:], OP.mult)
                nc.vector.tensor_scalar(out_sb[:, q4], num[:], b4_s[:, 0:1], None,
                                        OP.add)

            nc.sync.dma_start(out[:].rearrange("(q p) -> p q", p=128), out_sb[:])

    nc.compile()
    return nc


def make_in_maps(feat, coord, hr_guide, lr_guide,
                 W0, b0, W1, b1, W2, b2, W3, b3, W4, b4,
                 npoints=NP, ncores=NCORES):
    """Host-side shard + repack. Returns per-core input dicts."""
    f32 = np.float32
    W0 = np.asarray(W0, f32)
    w0a = np.ascontiguousarray(W0[0:128])
    w0b = np.ascontiguousarray(W0[128:256] + W0[256:384])
    w0c = np.ascontiguousarray(-W0[256:384])
    w0d = np.ascontiguousarray(W0[384:386])
    w1r = np.ascontiguousarray(
        np.asarray(W1, f32).reshape(8, 128, 512).transpose(1, 0, 2).reshape(128, 4096))
    w2r = np.ascontiguousarray(
        np.asarray(W2, f32).reshape(4, 128, 256).transpose(1, 0, 2).reshape(128, 1024))
    w3r = np.ascontiguousarray(
        np.asarray(W3, f32).reshape(2, 128, 128).transpose(1, 0, 2).reshape(128, 256))
    w4r = np.ascontiguousarray(np.asarray(W4, f32))
    b0r = np.ascontiguousarray(np.asarray(b0, f32).reshape(8, 128).T)
    b1r = np.ascontiguousarray(np.asarray(b1, f32).reshape(4, 128).T)
    b2r = np.ascontiguousarray(np.asarray(b2, f32).reshape(2, 128).T)
    b3r = np.ascontiguousarray(np.asarray(b3, f32).reshape(1, 128).T)
    b4r = np.full((128, 1), np.asarray(b4, f32)[0], f32)

    per_batch = {}
    for b in range(B):
        fl = np.concatenate([
            np.asarray(feat[b], f32).reshape(128, PIX_FL).T,
            np.asarray(lr_guide[b], f32).reshape(128, PIX_FL).T], axis=1)
        tfl = np.zeros((PIX_FL + 1, 256), f32)
        tfl[:PIX_FL] = fl
        thr = np.asarray(hr_guide[b], f32).reshape(128, PIX_HR).T
        per_batch[b] = (np.ascontiguousarray(tfl),
                        np.ascontiguousarray(thr[:PIX_HR // 2]),
                        np.ascontiguousarray(thr[PIX_HR // 2:]))

    halves = NFULL // npoints  # cores per batch
    in_maps = []
    for c in range(ncores):
        b = c // halves
        h = c % halves
        tfl, thr_lo, thr_hi = per_batch[b]
        cslice = np.ascontiguousarray(
            np.asarray(coord[b, h * npoints:(h + 1) * npoints], f32))
        in_maps.append({
            "tbl_fl": tfl, "tbl_hr_lo": thr_lo, "tbl_hr_hi": thr_hi,
            "coord": cslice,
            "w0a": w0a, "w0b": w0b, "w0c": w0c, "w0d": w0d,
            "w1": w1r, "w2": w2r, "w3": w3r, "w4": w4r,
            "bias0": b0r, "bias1": b1r, "bias2": b2r, "bias3": b3r,
            "bias4": b4r,
        })
    return in_maps


_CACHE = {}


def _get_program(npoints=NP):
    if npoints not in _CACHE:
        _CACHE[npoints] = build_program(npoints)
    return _CACHE[npoints]


def run_on_hw(inputs, trace=False):
    from concourse.bass_utils import run_bass_kernel_spmd
    nc = _get_program(NP)
    in_maps = make_in_maps(**inputs)
    res = run_bass_kernel_spmd(nc, in_maps, list(range(NCORES)), trace=trace)
    out = np.empty((B, NFULL, 1), np.float32)
    halves = NFULL // NP
    for c in range(NCORES):
        b, h = c // halves, c % halves
        out[b, h * NP:(h + 1) * NP, 0] = res.results[c]["out"]
    return out, res


def kernel(**inputs):
    out, _ = run_on_hw(inputs, trace=False)
    return out



# revision 4
# speedup vs baseline: 20.8747x; 20.8747x over previous
"""JIIF-style implicit-upsampling MLP on 8 Trainium2 NeuronCores.

Full inputs -> shard (batch, query-point) across 8 cores -> per-core Bass/Tile
kernel (gather + 5-layer MLP + softmax-combine) -> gather full output.

Per-core kernel design (NP = 32768 query points/core):
  * DRAM tables per core:  tbl_fl [4097, 256]  = concat(feat, lr_guide) per
    LR pixel + zero row (row 4096) for out-of-range zero padding;
    tbl_hr_lo/hi [32768, 128] = hr_guide per HR pixel, split in two halves
    because dma_gather indices are int16 (invalid hr points are zeroed via a
    mask multiply instead of a zero row).
  * On-device index math reproduces grid_sample_nearest bit-exactly
    (round-half-even via the 1.5*2^23 magic-add; invalid points redirected
    to the zero row / masked).
  * Gathers via gpsimd dma_gather, 512 rows per call (row i lands on
    partition i%128, slot i//128 — exactly the point-major layout); indices
    are int16 in the [16, n/16] wrap replicated across the 8 Q7 groups.
    (indirect_dma_start was abandoned: its compile-time DynamicDMA patching
    is disabled in this toolchain and returns wrong rows on HW.)
  * Gathered [pts, ch] tiles are PE-transposed to [ch, pts].
  * MLP (386-1024-512-256-128-2) as float32r matmuls, N=512 points per tile,
    PSUM K-accumulation, fused bias+ReLU evacuation alternating DVE/ACT.
  * Final layer uses activations as the stationary operand to produce
    [pts, 2] directly in PSUM; softmax over the 4 shifts + weighted sum is
    done in point-major layout on DVE/ACT.
"""
import sys

if "/opt/trn_rl_repo" not in sys.path:
    sys.path.insert(0, "/opt/trn_rl_repo")

import numpy as np

import concourse.bass as bass
import concourse.bacc as bacc
import concourse.tile as tile
from concourse import mybir
from concourse.masks import make_identity

F32 = mybir.dt.float32
F32R = mybir.dt.float32r
I32 = mybir.dt.int32
I16 = mybir.dt.int16
OP = mybir.AluOpType
ACTF = mybir.ActivationFunctionType
AX = mybir.AxisListType

MAGIC = 12582912.0  # 1.5 * 2**23 : forces round-to-nearest-even on f32 add

B, NFULL = 4, 65536
H_LR = 64
H_HR = 256
NCORES = 8
NP = (B * NFULL) // NCORES  # 32768 points per core
PIX_FL = H_LR * H_LR        # 4096
PIX_HR = H_HR * H_HR        # 65536
SHIFTS = [(-1.0 / 64, -1.0 / 64), (-1.0 / 64, 1.0 / 64),
          (1.0 / 64, -1.0 / 64), (1.0 / 64, 1.0 / 64)]


def build_program(npoints=NP, reps=1):
    """Build the per-core Bass program. npoints must be a multiple of 512.
    reps>1 repeats the whole compute (idempotent) for timing isolation."""
    assert npoints % 512 == 0
    NQ = npoints // 128          # free-dim length of point-major tiles
    T = NQ // 4                  # number of 512-point tiles

    nc = bacc.Bacc("TRN2", target_bir_lowering=False, debug=False)

    tbl_fl = nc.dram_tensor("tbl_fl", [PIX_FL + 1, 256], F32, kind="ExternalInput")
    tbl_hr_lo = nc.dram_tensor("tbl_hr_lo", [PIX_HR // 2, 128], F32, kind="ExternalInput")
    tbl_hr_hi = nc.dram_tensor("tbl_hr_hi", [PIX_HR // 2, 128], F32, kind="ExternalInput")
    coord = nc.dram_tensor("coord", [npoints, 2], F32, kind="ExternalInput")
    w0a = nc.dram_tensor("w0a", [128, 1024], F32R, kind="ExternalInput")
    w0b = nc.dram_tensor("w0b", [128, 1024], F32R, kind="ExternalInput")
    w0c = nc.dram_tensor("w0c", [128, 1024], F32R, kind="ExternalInput")
    w0d = nc.dram_tensor("w0d", [2, 1024], F32R, kind="ExternalInput")
    w1 = nc.dram_tensor("w1", [128, 4096], F32R, kind="ExternalInput")
    w2 = nc.dram_tensor("w2", [128, 1024], F32R, kind="ExternalInput")
    w3 = nc.dram_tensor("w3", [128, 256], F32R, kind="ExternalInput")
    w4 = nc.dram_tensor("w4", [128, 2], F32R, kind="ExternalInput")
    bias0 = nc.dram_tensor("bias0", [128, 8], F32, kind="ExternalInput")
    bias1 = nc.dram_tensor("bias1", [128, 4], F32, kind="ExternalInput")
    bias2 = nc.dram_tensor("bias2", [128, 2], F32, kind="ExternalInput")
    bias3 = nc.dram_tensor("bias3", [128, 1], F32, kind="ExternalInput")
    bias4 = nc.dram_tensor("bias4", [128, 1], F32, kind="ExternalInput")
    out = nc.dram_tensor("out", [npoints], F32, kind="ExternalOutput")

    evac_ctr = [0]

    def evac_relu(dst, src, bias_ap):
        # relu(src + bias), alternating DVE / ACT to balance engine load
        if evac_ctr[0] % 2 == 0:
            nc.vector.tensor_scalar(dst, src, bias_ap, 0.0, OP.add, OP.max)
        else:
            nc.scalar.activation(dst, src, ACTF.Relu, bias=bias_ap, scale=1.0)
        evac_ctr[0] += 1

    def evac_copy(dst, src):
        if evac_ctr[0] % 2 == 0:
            nc.vector.tensor_copy(dst, src)
        else:
            nc.scalar.copy(dst, src)
        evac_ctr[0] += 1

    with tile.TileContext(nc) as tc:
        with tc.tile_pool(name="const", bufs=1) as cp, \
             tc.tile_pool(name="prol", bufs=1) as pp, \
             tc.tile_pool(name="gat", bufs=3) as gp, \
             tc.tile_pool(name="rhs", bufs=3) as rp, \
             tc.tile_pool(name="act", bufs=2) as ap, \
             tc.tile_pool(name="sm", bufs=2) as smp, \
             tc.tile_pool(name="ps", bufs=1, space="PSUM") as ps:

            ident = cp.tile([128, 128], F32)
            make_identity(nc, ident[:])

            # ---- load weights / biases ----
            w0a_s = cp.tile([128, 1024], F32R)
            w0b_s = cp.tile([128, 1024], F32R)
            w0c_s = cp.tile([128, 1024], F32R)
            w0d_s = cp.tile([2, 1024], F32R)
            w1_s = cp.tile([128, 4096], F32R)
            w2_s = cp.tile([128, 1024], F32R)
            w3_s = cp.tile([128, 256], F32R)
            w4_s = cp.tile([128, 2], F32R)
            b0_s = cp.tile([128, 8], F32)
            b1_s = cp.tile([128, 4], F32)
            b2_s = cp.tile([128, 2], F32)
            b3_s = cp.tile([128, 1], F32)
            b4_s = cp.tile([128, 1], F32)
            for dst, src in [(w0a_s, w0a), (w0b_s, w0b), (w0c_s, w0c),
                             (w0d_s, w0d), (w1_s, w1), (w2_s, w2), (w3_s, w3),
                             (w4_s, w4), (b0_s, bias0), (b1_s, bias1),
                             (b2_s, bias2), (b3_s, bias3), (b4_s, bias4)]:
                nc.sync.dma_start(dst[:], src[:])

            # ---- load coords: point n -> (partition n%128, free n//128) ----
            C = pp.tile([128, NQ, 2], F32)
            nc.sync.dma_start(C[:], coord[:].rearrange("(q p) t -> p q t", p=128))

            # ---- index math ----
            def axis_index(c_ap, shift, Hval, tag):
                """Returns (rc, m): clipped rounded index + valid mask, [128, NQ] f32."""
                t0 = pp.tile([128, NQ], F32, tag="ax_t")
                if shift is not None:
                    nc.vector.tensor_scalar(t0[:], c_ap, shift, None, OP.add)
                    src = t0[:]
                else:
                    src = c_ap
                v = pp.tile([128, NQ], F32, tag="ax_v")
                nc.vector.tensor_scalar(v[:], src, 1.0, float(Hval), OP.add, OP.mult)
                nc.vector.tensor_scalar(v[:], v[:], 1.0, 0.5, OP.subtract, OP.mult)
                r = pp.tile([128, NQ], F32, tag="ax_r")
                nc.vector.tensor_scalar(r[:], v[:], MAGIC, MAGIC, OP.add, OP.subtract)
                rc = pp.tile([128, NQ], F32, tag=tag[-1] + "_rc")
                nc.vector.tensor_scalar(rc[:], r[:], 0.0, float(Hval - 1), OP.max, OP.min)
                m = pp.tile([128, NQ], F32, tag=tag[-1] + "_m")
                nc.vector.tensor_tensor(m[:], r[:], rc[:], OP.is_equal)
                return rc, m

            def lin_index(ry, rx, my, mx, Hval, tag, redirect=True):
                """Combined mask + linear index; optionally redirected to the
                zero row (Hval^2) when invalid."""
                m = pp.tile([128, NQ], F32, tag="li_mm")
                nc.vector.tensor_tensor(m[:], my[:], mx[:], OP.mult)
                idx = pp.tile([128, NQ], F32, tag="li_idx")
                nc.vector.scalar_tensor_tensor(idx[:], ry[:], float(Hval), rx[:],
                                               OP.mult, OP.add)
                if redirect:
                    zr = float(Hval * Hval)
                    nc.vector.scalar_tensor_tensor(idx[:], idx[:], -zr, m[:],
                                                   OP.add, OP.mult)
                    nc.vector.tensor_scalar(idx[:], idx[:], zr, None, OP.add)
                return idx, m

            def wrap16(src_i16, tag):
                # relayout [128, NQ] -> dma_gather's [16, NQ*8] wrap,
                # replicated across the 8 Q7 partition groups
                wr = pp.tile([128, NQ * 8], I16, tag=tag + "_wr")
                for ph in range(8):
                    nc.sync.dma_start(wr[0:16, ph::8],
                                      src_i16[ph * 16:(ph + 1) * 16, :])
                for rep in range(1, 8):
                    nc.sync.dma_start(wr[rep * 16:(rep + 1) * 16, :], wr[0:16, :])
                return wr

            cy, cx = C[:, :, 0], C[:, :, 1]

            ry_h, my_h = axis_index(cy, None, H_HR, "hy")
            rx_h, mx_h = axis_index(cx, None, H_HR, "hx")
            idx_hf, m_hr = lin_index(ry_h, rx_h, my_h, mx_h, H_HR, "h",
                                     redirect=False)
            HALF = float(PIX_HR // 2)
            hi_m = pp.tile([128, NQ], F32)
            nc.vector.tensor_scalar(hi_m[:], idx_hf[:], HALF, None, OP.is_ge)
            one_m_hi = pp.tile([128, NQ], F32)
            nc.vector.tensor_scalar(one_m_hi[:], hi_m[:], -1.0, 1.0, OP.mult, OP.add)
            ilo = pp.tile([128, NQ], F32)
            nc.vector.tensor_tensor(ilo[:], idx_hf[:], one_m_hi[:], OP.mult)
            ihi = pp.tile([128, NQ], F32)
            nc.vector.scalar_tensor_tensor(ihi[:], idx_hf[:], -HALF, hi_m[:],
                                           OP.add, OP.mult)
            mlo_m = pp.tile([128, NQ], F32)
            nc.vector.tensor_tensor(mlo_m[:], one_m_hi[:], m_hr[:], OP.mult)
            mhi_m = pp.tile([128, NQ], F32)
            nc.vector.tensor_tensor(mhi_m[:], hi_m[:], m_hr[:], OP.mult)
            ilo16 = pp.tile([128, NQ], I16)
            nc.vector.tensor_copy(ilo16[:], ilo[:])
            ihi16 = pp.tile([128, NQ], I16)
            nc.vector.tensor_copy(ihi16[:], ihi[:])
            wr_hlo = wrap16(ilo16, "hlo")
            wr_hhi = wrap16(ihi16, "hhi")

            idx_fl = []
            rel = []
            for s, (sy, sx) in enumerate(SHIFTS):
                ry, my = axis_index(cy, sy, H_LR, "fy")
                rx, mx = axis_index(cx, sx, H_LR, "fx")
                fidx, m = lin_index(ry, rx, my, mx, H_LR, f"f{s}")
                f16 = pp.tile([128, NQ], I16, tag="f16")
                nc.vector.tensor_copy(f16[:], fidx[:])
                idx_fl.append(wrap16(f16, f"fw{s}"))
                # rel_coord = (coord - valid*pix_coord) * 64, bit-exact vs ref
                rl = pp.tile([128, NQ, 2], F32, tag=f"rel{s}")
                for comp, (rc_c, c_c) in enumerate([(ry, cy), (rx, cx)]):
                    qc = pp.tile([128, NQ], F32, tag="qc")
                    nc.vector.tensor_scalar(qc[:], rc_c[:], 0.03125, -0.984375,
                                            OP.mult, OP.add)
                    nc.vector.tensor_tensor(qc[:], qc[:], m[:], OP.mult)
                    nc.vector.tensor_tensor(qc[:], c_c, qc[:], OP.subtract)
                    nc.vector.tensor_scalar(rl[:, :, comp], qc[:], 64.0, None, OP.mult)
                rel.append(rl)

            out_sb = pp.tile([128, NQ], F32)

            # ---- main loop over 512-point tiles ----
            for t in [tt for _ in range(reps) for tt in range(T)]:
                q4 = slice(t * 4, t * 4 + 4)

                # hr gather + transpose -> hrT [128ch, 512pts]
                gh = gp.tile([128, 4, 128], F32, tag="gh")
                ghi = gp.tile([128, 4, 128], F32, tag="ghi")
                w32 = slice(t * 32, (t + 1) * 32)
                nc.gpsimd.dma_gather(gh[:], tbl_hr_lo[:], wr_hlo[:, w32],
                                     num_idxs=512, num_idxs_reg=512, elem_size=128)
                nc.gpsimd.dma_gather(ghi[:], tbl_hr_hi[:], wr_hhi[:, w32],
                                     num_idxs=512, num_idxs_reg=512, elem_size=128)
                mlo_b = mlo_m[:, q4].unsqueeze(2).to_broadcast([128, 4, 128])
                mhi_b = mhi_m[:, q4].unsqueeze(2).to_broadcast([128, 4, 128])
                nc.vector.tensor_tensor(gh[:], gh[:], mlo_b, OP.mult)
                nc.vector.tensor_tensor(ghi[:], ghi[:], mhi_b, OP.mult)
                nc.vector.tensor_tensor(gh[:], gh[:], ghi[:], OP.add)
                pt_h = ps.tile([128, 512], F32, tag="pt", bufs=3)
                for q in range(4):
                    nc.tensor.transpose(pt_h[:, q * 128:(q + 1) * 128],
                                        gh[:, q, :], ident[:])
                hrT = rp.tile([128, 512], F32R, tag="hrT")
                evac_copy(hrT[:], pt_h[:])

                p4 = ps.tile([128, 32], F32, tag="p4", bufs=2)

                for s in range(4):
                    gfl = gp.tile([128, 4, 256], F32, tag="gfl")
                    nc.gpsimd.dma_gather(gfl[:], tbl_fl[:], idx_fl[s][:, w32],
                                         num_idxs=512, num_idxs_reg=512,
                                         elem_size=256)

                    pt_f = ps.tile([128, 512], F32, tag="pt", bufs=3)
                    pt_l = ps.tile([128, 512], F32, tag="pt", bufs=3)
                    for q in range(4):
                        nc.tensor.transpose(pt_f[:, q * 128:(q + 1) * 128],
                                            gfl[:, q, 0:128], ident[:])
                        nc.tensor.transpose(pt_l[:, q * 128:(q + 1) * 128],
                                            gfl[:, q, 128:256], ident[:])
                    featT = rp.tile([128, 512], F32R, tag="featT")
                    lrT = rp.tile([128, 512], F32R, tag="lrT")
                    evac_copy(featT[:], pt_f[:])
                    evac_copy(lrT[:], pt_l[:])

                    pt_r = ps.tile([2, 512], F32, tag="pt", bufs=3)
                    for q in range(4):
                        nc.tensor.transpose(pt_r[:, q * 128:(q + 1) * 128],
                                            rel[s][:, t * 4 + q, :], ident[:])
                    relT = rp.tile([2, 512], F32R, tag="relT")
                    evac_copy(relT[:], pt_r[:])

                    # L0: 386 -> 1024
                    a0 = ap.tile([128, 8, 512], F32R, tag="a0", bufs=1)
                    for m in range(8):
                        ms = slice(m * 128, (m + 1) * 128)
                        p0 = ps.tile([128, 512], F32, tag="pmm", bufs=3)
                        nc.tensor.matmul(p0[:], w0a_s[:, ms], featT[:],
                                         start=True, stop=False)
                        nc.tensor.matmul(p0[:], w0b_s[:, ms], hrT[:],
                                         start=False, stop=False)
                        nc.tensor.matmul(p0[:], w0c_s[:, ms], lrT[:],
                                         start=False, stop=False)
                        nc.tensor.matmul(p0[:], w0d_s[:, ms], relT[:],
                                         start=False, stop=True)
                        evac_relu(a0[:, m, :], p0[:], b0_s[:, m:m + 1])

                    # L1: 1024 -> 512
                    a1 = ap.tile([128, 4, 512], F32R, tag="a1")
                    for m in range(4):
                        p1 = ps.tile([128, 512], F32, tag="pmm", bufs=3)
                        for k in range(8):
                            nc.tensor.matmul(
                                p1[:],
                                w1_s[:, k * 512 + m * 128: k * 512 + (m + 1) * 128],
                                a0[:, k, :],
                                start=(k == 0), stop=(k == 7))
                        evac_relu(a1[:, m, :], p1[:], b1_s[:, m:m + 1])

                    # L2: 512 -> 256
                    a2 = ap.tile([128, 2, 512], F32R, tag="a2")
                    for m in range(2):
                        p2 = ps.tile([128, 512], F32, tag="pmm", bufs=3)
                        for k in range(4):
                            nc.tensor.matmul(
                                p2[:],
                                w2_s[:, k * 256 + m * 128: k * 256 + (m + 1) * 128],
                                a1[:, k, :],
                                start=(k == 0), stop=(k == 3))
                        evac_relu(a2[:, m, :], p2[:], b2_s[:, m:m + 1])

                    # L3: 256 -> 128
                    a3 = ap.tile([128, 512], F32R, tag="a3")
                    p3 = ps.tile([128, 512], F32, tag="pmm", bufs=3)
                    for k in range(2):
                        nc.tensor.matmul(p3[:],
                                         w3_s[:, k * 128:(k + 1) * 128],
                                         a2[:, k, :],
                                         start=(k == 0), stop=(k == 1))
                    evac_relu(a3[:], p3[:], b3_s[:, 0:1])

                    # L4: 128 -> 2, activations stationary -> [pts, 2] in PSUM
                    for q in range(4):
                        off = (q * 4 + s) * 2
                        nc.tensor.matmul(p4[:, off:off + 2],
                                         a3[:, q * 128:(q + 1) * 128],
                                         w4_s[:],
                                         start=True, stop=True)

                # softmax over shifts + weighted sum (point-major layout)
                p4v = p4[:].rearrange("p (q s c) -> p q s c", q=4, s=4)
                mx = smp.tile([128, 4], F32, tag="mx")
                nc.vector.tensor_reduce(mx[:], p4v[:, :, :, 1], AX.X, OP.max)
                e = smp.tile([128, 4, 4], F32, tag="e")
                mxb = mx[:].unsqueeze(2).to_broadcast([128, 4, 4])
                nc.vector.tensor_tensor(e[:], p4v[:, :, :, 1], mxb, OP.subtract)
                nc.scalar.activation(e[:], e[:], ACTF.Exp)
                ssum = smp.tile([128, 4], F32, tag="ssum")
                nc.vector.tensor_reduce(ssum[:], e[:], AX.X, OP.add)
                nc.vector.tensor_tensor(e[:], e[:], p4v[:, :, :, 0], OP.mult)
                num = smp.tile([128, 4], F32, tag="num")
                nc.vector.tensor_reduce(num[:], e[:], AX.X, OP.add)
                rec = smp.tile([128, 4], F32, tag="rec")
                nc.vector.reciprocal(rec[:], ssum[:])
                nc.vector.tensor_tensor(num[:], num[:], rec[:], OP.mult)
                nc.vector.tensor_scalar(out_sb[:, q4], num[:], b4_s[:, 0:1], None,
                                        OP.add)

            nc.sync.dma_start(out[:].rearrange("(q p) -> p q", p=128), out_sb[:])

    nc.compile()
    return nc


def make_in_maps(feat, coord, hr_guide, lr_guide,
                 W0, b0, W1, b1, W2, b2, W3, b3, W4, b4,
                 npoints=NP, ncores=NCORES):
    """Host-side shard + repack. Returns per-core input dicts."""
    f32 = np.float32
    W0 = np.asarray(W0, f32)
    w0a = np.ascontiguousarray(W0[0:128])
    w0b = np.ascontiguousarray(W0[128:256] + W0[256:384])
    w0c = np.ascontiguousarray(-W0[256:384])
    w0d = np.ascontiguousarray(W0[384:386])
    w1r = np.ascontiguousarray(
        np.asarray(W1, f32).reshape(8, 128, 512).transpose(1, 0, 2).reshape(128, 4096))
    w2r = np.ascontiguousarray(
        np.asarray(W2, f32).reshape(4, 128, 256).transpose(1, 0, 2).reshape(128, 1024))
    w3r = np.ascontiguousarray(
        np.asarray(W3, f32).reshape(2, 128, 128).transpose(1, 0, 2).reshape(128, 256))
    w4r = np.ascontiguousarray(np.asarray(W4, f32))
    b0r = np.ascontiguousarray(np.asarray(b0, f32).reshape(8, 128).T)
    b1r = np.ascontiguousarray(np.asarray(b1, f32).reshape(4, 128).T)
    b2r = np.ascontiguousarray(np.asarray(b2, f32).reshape(2, 128).T)
    b3r = np.ascontiguousarray(np.asarray(b3, f32).reshape(1, 128).T)
    b4r = np.full((128, 1), np.asarray(b4, f32)[0], f32)

    per_batch = {}
    for b in range(B):
        fl = np.concatenate([
            np.asarray(feat[b], f32).reshape(128, PIX_FL).T,
            np.asarray(lr_guide[b], f32).reshape(128, PIX_FL).T], axis=1)
        tfl = np.zeros((PIX_FL + 1, 256), f32)
        tfl[:PIX_FL] = fl
        thr = np.asarray(hr_guide[b], f32).reshape(128, PIX_HR).T
        per_batch[b] = (np.ascontiguousarray(tfl),
                        np.ascontiguousarray(thr[:PIX_HR // 2]),
                        np.ascontiguousarray(thr[PIX_HR // 2:]))

    halves = NFULL // npoints  # cores per batch
    in_maps = []
    for c in range(ncores):
        b = c // halves
        h = c % halves
        tfl, thr_lo, thr_hi = per_batch[b]
        cslice = np.ascontiguousarray(
            np.asarray(coord[b, h * npoints:(h + 1) * npoints], f32))
        in_maps.append({
            "tbl_fl": tfl, "tbl_hr_lo": thr_lo, "tbl_hr_hi": thr_hi,
            "coord": cslice,
            "w0a": w0a, "w0b": w0b, "w0c": w0c, "w0d": w0d,
            "w1": w1r, "w2": w2r, "w3": w3r, "w4": w4r,
            "bias0": b0r, "bias1": b1r, "bias2": b2r, "bias3": b3r,
            "bias4": b4r,
        })
    return in_maps


_CACHE = {}


def _get_program(npoints=NP, reps=1):
    key = (npoints, reps)
    if key not in _CACHE:
        _CACHE[key] = build_program(npoints, reps)
    return _CACHE[key]


def run_on_hw(inputs, trace=False):
    from concourse.bass_utils import run_bass_kernel_spmd
    nc = _get_program(NP)
    in_maps = make_in_maps(**inputs)
    res = run_bass_kernel_spmd(nc, in_maps, list(range(NCORES)), trace=trace)
    out = np.empty((B, NFULL, 1), np.float32)
    halves = NFULL // NP
    for c in range(NCORES):
        b, h = c // halves, c % halves
        out[b, h * NP:(h + 1) * NP, 0] = res.results[c]["out"]
    return out, res


def kernel(**inputs):
    out, _ = run_on_hw(inputs, trace=False)
    return out

